# revision 36
# baseline (speedup 1.0000x reference)
"""Causal self-attention (B=2,T=2048,C=1024,H=16) on 8 trn2 NeuronCores.

Sharding: core c handles batch b=c//4 and 4 heads (c%4)*4..+4 (tensor-parallel
over heads x data-parallel over batch).

Per-core structure: one software-pipelined stream of work units on the PE,
interleaved so the scalar engine (exp) and the vector engines (bias moves,
normalize) are never the rate limiter for long:

  phase k in 0..3:
    [qk-proj mop0 | attV(k-1) head-pair 0]
    [scores(k) hp0 units  x  {qk mop1, V tp0, attV(k-1) hp1, outproj(k-2) a}]
    [scores(k) hp1 units  x  {V tp1, outproj(k-2) b}]
  tail: attV(3) + outproj(2) + per-subtile outproj(3)

Layouts:
  - scores S^T = K_tile' Q_chunk [k-pos, q] in PSUM; exp -> bf16 es in SBUF,
    causally trimmed; triangular mask via gpsimd affine_select on diagonal
    subtiles only.
  - attV FLIPPED: stationary = es [128k x 128q] block, moving = V tile
    [128k, 65] bf16 (64 ch + ones col -> softmax denominator in col 64).
    Out y = [q, 65] per (head, q-subtile): 65-cycle matmuls.
  - normalize: rc = 1/l per partition (DVE), y*rc -> bf16 [q, 2*64ch] (Pool),
    XBAR dma-transpose -> yT [ch, q] bf16 (no PE transpose).
  - out-proj: stationary W_proj f32r, moving yT bf16; b_proj folded into the
    PSUM->SBUF move (DVE/Pool alternating).
"""
import sys

sys.path.insert(0, "/opt/trn_rl_repo")

import numpy as np

import concourse.bass as bass
import concourse.mybir as mybir
import concourse.tile as tile
from concourse import bacc
from concourse.bass_utils import run_bass_kernel_spmd

B, T, C, H, HD = 2, 2048, 1024, 16, 64
NCORES = 8
HPC = 4            # heads per core
CT = C // 128      # 8 contraction tiles
TJ = T // 512      # 4 tok chunks
TT = T // 128      # 16 tok tiles
VW = HPC * (HD + 1)  # 260: V cols per core incl. ones column per head
F32 = mybir.dt.float32
F32R = mybir.dt.float32r
BF16 = mybir.dt.bfloat16
EXP = mybir.ActivationFunctionType.Exp

_CACHE = {}

# emission-structure knobs (swept experimentally; cost model is the metric)
CFG = {
    "order": "wmix",      # "blocks" = sequential phase blocks, "wmix" = paced interleave
    "attv_pre": "both",    # "hp0": attV hp1 goes into fillers; "both": all attV in pre-block
    "norm_eng": "vector",  # engine for the normalize multiply
    "oproj_eng": "pool",   # engine(s) for out-proj PSUM->SBUF moves
}


def _wmix(s_units, f_units):
    """Spread weighted fillers among score units by cumulative PE time.

    s_units: list of closures; f_units: list of (closure, pe_ns). After each
    score unit, emit fillers until their cumulative time reaches the uniform
    per-score share. Keeps the exp engine fed at a steady cadence.
    """
    if not s_units:
        return [f for f, _ in f_units]
    total = sum(w for _, w in f_units)
    out, fi, acc, target = [], 0, 0.0, 0.0
    for s in s_units:
        out.append(s)
        target += total / len(s_units)
        while fi < len(f_units) and acc < target - 1e-9:
            out.append(f_units[fi][0])
            acc += f_units[fi][1]
            fi += 1
    out.extend(f for f, _ in f_units[fi:])
    return out


def _emit(tc, nc, d):
    d_xT, d_wqk, d_wv, d_wp, d_bqk, d_bv, d_bp, d_tri, d_out = d
    d_xT3 = d_xT.rearrange("p (c t) -> p c t", c=CT)
    d_out3 = d_out.rearrange("p (m t) -> p m t", m=8)

    with tc.tile_pool(name="const", bufs=1) as pc, \
         tc.tile_pool(name="w", bufs=1) as pw, \
         tc.tile_pool(name="x", bufs=2) as px, \
         tc.tile_pool(name="qk", bufs=1) as pqk, \
         tc.tile_pool(name="vv", bufs=1) as pvv, \
         tc.tile_pool(name="es", bufs=1) as pes, \
         tc.tile_pool(name="yt", bufs=1) as pyt, \
         tc.tile_pool(name="yb", bufs=4) as pyb, \
         tc.tile_pool(name="rc", bufs=4) as prc, \
         tc.tile_pool(name="o", bufs=3) as po, \
         tc.tile_pool(name="o3", bufs=1) as po3, \
         tc.tile_pool(name="psS", bufs=2, space="PSUM") as psS, \
         tc.tile_pool(name="psY", bufs=2, space="PSUM") as psY, \
         tc.tile_pool(name="psC", bufs=2, space="PSUM") as psC:

        bqk = pc.tile([128, 4], F32, tag="bqk")
        tri = pc.tile([128, 128], BF16, tag="tri")
        bv = pc.tile([128, VW], F32, tag="bv")
        bp = pc.tile([128, 8], F32, tag="bp")
        wp = pc.tile([128, 2 * C], BF16, tag="wp")
        wqk = pw.tile([128, CT * 512], BF16, tag="wqk")
        wv = pw.tile([128, CT * VW], BF16, tag="wv")

        qkT = [pqk.tile([128, T], F32R, tag=f"qk{i}", name=f"qkT{i}")
               for i in range(4)]
        V = [pvv.tile([128, VW], BF16, tag=f"v{i}", name=f"V{i}")
             for i in range(TT)]
        yT = [pyt.tile([128, T], BF16, tag=f"y{i}", name=f"yT{i}")
              for i in range(2)]

        # ---- preamble DMAs: wqk + x(tj0) interleaved ----
        xt = [None] * TJ
        xt[0] = px.tile([128, CT * 512], BF16, tag="xt", name="xt0")
        xt0_3 = xt[0].rearrange("p (c t) -> p c t", c=CT)
        for ct in range(2):
            nc.sync.dma_start(wqk[:, ct * 512:(ct + 1) * 512],
                              d_wqk[:, ct * 512:(ct + 1) * 512])
            nc.sync.dma_start(xt0_3[:, ct:ct + 1, :],
                              d_xT3[:, ct:ct + 1, 0:512])
        for cp in range(1, 4):
            nc.sync.dma_start(wqk[:, cp * 1024:(cp + 1) * 1024],
                              d_wqk[:, cp * 1024:(cp + 1) * 1024])
            nc.sync.dma_start(xt0_3[:, 2 * cp:2 * cp + 2, :],
                              d_xT3[:, 2 * cp:2 * cp + 2, 0:512])
        xt_preloaded = True
        nc.sync.dma_start(bqk[:], d_bqk)
        nc.sync.dma_start(tri[:], d_tri)
        for h2 in range(2):
            nc.sync.dma_start(wv[:, h2 * 4 * VW:(h2 + 1) * 4 * VW],
                              d_wv[:, h2 * 4 * VW:(h2 + 1) * 4 * VW])
        nc.sync.dma_start(bv[:], d_bv)
        nc.sync.dma_start(wp[:], d_wp)
        nc.sync.dma_start(bp[:], d_bp)

        es_h = {}   # (head, pair) -> es tile of the current chunk
        nmove = [0]  # rotating engine picker for PSUM->SBUF moves

    # ---------------- unit builders (each returns a closure) -------------
        def u_qk(k, mo):
            def run():
                s = psC.tile([128, 512], F32, tag="pp")
                for ct in range(CT):
                    nc.tensor.matmul(
                        s[:],
                        wqk[:, ct * 512 + mo * 128:ct * 512 + (mo + 1) * 128],
                        xt[k][:, ct * 512:(ct + 1) * 512],
                        start=(ct == 0), stop=(ct == CT - 1))
                nc.vector.tensor_scalar_add(
                    qkT[mo][:, k * 512:(k + 1) * 512], s[:], bqk[:, mo:mo + 1])
            return run

        def u_v(k, lt):
            def run():
                s = psC.tile([128, 512], F32, tag="pp")
                for ct in range(CT):
                    nc.tensor.matmul(
                        s[:, 0:VW],
                        xt[k][:, ct * 512 + lt * 128:ct * 512 + (lt + 1) * 128],
                        wv[:, ct * VW:(ct + 1) * VW],
                        start=(ct == 0), stop=(ct == CT - 1))
                nc.vector.tensor_add(V[4 * k + lt][:], s[:, 0:VW], bv[:])
            return run

        def u_score(qj, hp, p, lh):
            def run():
                h = 2 * hp + lh
                lo = 64 * lh
                qt_t, kt_t = qkT[2 * hp], qkT[2 * hp + 1]
                s = psS.tile([128, 1024], F32, tag="s")
                par = qj % 2 if h < 2 else 0
                es = pes.tile([128, 1024], BF16, tag=f"es{h}_{p}_{par}",
                              name=f"es{h}_{p}_{qj}")
                es_h[(h, p, par)] = es
                for ki in range(2):
                    kti = 2 * p + ki
                    dg = kti - 4 * qj
                    so = 0 if dg < 0 else min(dg, 2) * 128
                    nc.tensor.matmul(
                        s[:, ki * 512 + so:(ki + 1) * 512],
                        kt_t[lo:lo + 64, kti * 128:(kti + 1) * 128],
                        qt_t[lo:lo + 64, qj * 512 + so:(qj + 1) * 512],
                        start=True, stop=True)
                if 2 * p + 1 < 4 * qj:
                    nc.scalar.activation(es[:], s[:], EXP)
                else:
                    for ki in range(2):
                        kti = 2 * p + ki
                        dg = kti - 4 * qj
                        eo = max(dg, 0) * 128
                        nc.scalar.activation(
                            es[:, ki * 512 + eo:(ki + 1) * 512],
                            s[:, ki * 512 + eo:(ki + 1) * 512], EXP)
                        if dg >= 0:
                            nc.gpsimd.affine_select(
                                out=es[:, ki * 512 + dg * 128:
                                       ki * 512 + (dg + 1) * 128],
                                in_=es[:, ki * 512 + dg * 128:
                                       ki * 512 + (dg + 1) * 128],
                                compare_op=mybir.AluOpType.is_ge,
                                fill=0.0, base=0,
                                pattern=[[1, 128]],
                                channel_multiplier=-1)
            return run

        def u_attv(qj, sub):
            def run():
                qt_g = 4 * qj + sub
                yp = psY.tile([128, VW], F32, tag="yp")
                for h in range(4):
                    par = qj % 2 if h < 2 else 0
                    for kti in range(qt_g + 1):
                        p, ki = divmod(kti, 2)
                        nc.tensor.matmul(
                            yp[:, h * 65:h * 65 + 65],
                            es_h[(h, p, par)][:, ki * 512 + sub * 128:
                                              ki * 512 + (sub + 1) * 128],
                            V[kti][:, h * 65:h * 65 + 65],
                            start=(kti == 0), stop=(kti == qt_g))
                rc = prc.tile([128, 4], F32, tag="rc")
                nc.vector.reciprocal(rc[:], yp[:, 64:VW:65])
                yb = pyb.tile([128, 256], BF16, tag="yb")
                # one broadcast multiply: [128, 4head x 64ch] x rc (stride-0)
                yp4 = bass.AP(yp.tensor, yp.offset,
                              [[VW, 128], [65, 4], [1, 64]])
                rcb = bass.AP(rc.tensor, rc.offset,
                              [[4, 128], [1, 4], [0, 64]])
                nc.vector.tensor_mul(
                    yb.rearrange("p (h c) -> p h c", h=4), yp4, rcb)
                for hp2 in range(2):
                    nc.sync.dma_start_transpose(
                        yT[hp2][:, qt_g * 128:(qt_g + 1) * 128],
                        yb[:, hp2 * 128:(hp2 + 1) * 128])
            return run

        def u_oproj(qj, mop):
            def run():
                ot = po.tile([128, 1024], BF16, tag="ot")
                for half in range(2):
                    mo = 2 * mop + half
                    pp = psC.tile([128, 512], F32, tag="pp")
                    for kt2 in range(2):
                        nc.tensor.matmul(
                            pp[:],
                            wp[:, kt2 * C + mo * 128:kt2 * C + (mo + 1) * 128],
                            yT[kt2][:, qj * 512:(qj + 1) * 512],
                            start=(kt2 == 0), stop=(kt2 == 1))
                    nc.vector.tensor_scalar_add(
                        ot[:, half * 512:(half + 1) * 512],
                        pp[:], bp[:, mo:mo + 1])
                ot3 = ot.rearrange("p (m t) -> p m t", m=2)
                nc.sync.dma_start(
                    d_out3[:, 2 * mop:2 * mop + 2, qj * 512:(qj + 1) * 512],
                    ot3)
            return run

        # per-subtile out-proj for the last chunk (shrinks the tail)
        ot3_tiles = {}

        def u_oproj3_sub(sub):
            def run():
                qj = TJ - 1
                for mop in range(4):
                    if sub == 0:
                        ot3_tiles[mop] = po3.tile([128, 1024], BF16,
                                                  tag=f"ot3_{mop}",
                                                  name=f"ot3_{mop}")
                    ot = ot3_tiles[mop]
                    pp = psC.tile([128, 512], F32, tag="pp")
                    for half in range(2):
                        mo = 2 * mop + half
                        for kt2 in range(2):
                            nc.tensor.matmul(
                                pp[:, half * 128:(half + 1) * 128],
                                wp[:, kt2 * C + mo * 128:kt2 * C + (mo + 1) * 128],
                                yT[kt2][:, (4 * qj + sub) * 128:
                                        (4 * qj + sub + 1) * 128],
                                start=(kt2 == 0), stop=(kt2 == 1))
                    otv = bass.AP(ot.tensor,
                                  ot.offset + sub * 128,
                                  [[1024, 128], [512, 2], [1, 128]])
                    ppv = pp[:, 0:256].rearrange("p (h c) -> p h c", h=2)
                    bpv = bass.AP(bp.tensor, bp.offset + 2 * mop,
                                  [[8, 128], [1, 2], [0, 128]])
                    nc.vector.tensor_add(otv, ppv, bpv)
                    if sub % 2 == 1:
                        ot3 = ot.rearrange("p (m t) -> p m t", m=2)
                        qa = qj * 512 + (sub - 1) * 128
                        nc.sync.dma_start(
                            d_out3[:, 2 * mop:2 * mop + 2, qa:qa + 256],
                            ot3[:, :, (sub - 1) * 128:(sub + 1) * 128])
            return run

        # ---------------- the phase stream ----------------
        def dma_xt(k):
            if k >= TJ or xt[k] is not None:
                return
            xt[k] = px.tile([128, CT * 512], BF16, tag="xt", name=f"xt{k}x")
            xtn3 = xt[k].rearrange("p (c t) -> p c t", c=CT)
            for cp in range(4):
                nc.sync.dma_start(
                    xtn3[:, 2 * cp:2 * cp + 2, :],
                    d_xT3[:, 2 * cp:2 * cp + 2, k * 512:(k + 1) * 512])

        for k in range(TJ):

            W_QK, W_V, W_OP = 1707.0, 867.0, 854.0

            def w_attv(qj, sub):
                return 108.4 * (4 * qj + sub + 1)

            s0 = [u_score(k, 0, p, lh)
                  for p in range(2 * k + 2) for lh in range(2)]
            s1 = [u_score(k, 1, p, lh)
                  for p in range(2 * k + 2) for lh in range(2)]
            # attV(k, hp0) is ready once s0 completes: it fills s1.
            # attV(k, hp1) becomes the next phase's pre-block.
            fill0 = []
            if k == 0:
                for u in [u_qk(0, 0), u_qk(0, 1)]:
                    u()
                fill0 += [(u_qk(0, 2), W_QK), (u_qk(0, 3), W_QK),
                          (u_v(0, 0), W_V), (u_v(0, 1), W_V)]
            if k >= 1:
                fill0 += [(u_attv(k - 1, s), w_attv(k - 1, s))
                          for s in range(4)]
            fill0 += [(u_v(k, 2), W_V), (u_v(k, 3), W_V)]
            fill1 = []
            nxt = ([(u_qk(k + 1, mo), W_QK) for mo in range(4)]
                   + [(u_v(k + 1, 0), W_V), (u_v(k + 1, 1), W_V)]
                   if k < TJ - 1 else [])
            ops = ([(u_oproj(k - 1, mop), W_OP) for mop in range(4)]
                   if k >= 1 else [])
            # spread oproj among next-phase fillers, oproj not first
            while nxt or ops:
                if len(nxt) >= len(ops) and nxt:
                    fill1.append(nxt.pop(0))
                elif ops:
                    fill1.append(ops.pop(0))

            dma_xt(k + 1)
            for u in _wmix(s0, fill0):
                u()
            tail_av = []
            if k == TJ - 1:
                # attV(3,0/1) right after the s1 units they depend on
                mix = _wmix(s1, fill1)
                n_dep0 = 0
                seen = 0
                out_units = []
                for u in mix:
                    out_units.append(u)
                for u in out_units:
                    u()
                u_attv(k, 0)()
                u_attv(k, 1)()
            else:
                for u in _wmix(s1, fill1):
                    u()

        # ---------------- tail ----------------
        for u in [u_attv(TJ - 1, 2), u_oproj3_sub(0),
                  u_attv(TJ - 1, 3), u_oproj3_sub(1),
                  u_oproj3_sub(2), u_oproj3_sub(3)]:
            u()


def _build(reps=1):
    nc = bacc.Bacc("TRN2", target_bir_lowering=False, debug=False)
    d = (
        nc.dram_tensor("xT", [128, CT * T], BF16, kind="ExternalInput").ap(),
        nc.dram_tensor("wqk", [128, CT * 512], BF16, kind="ExternalInput").ap(),
        nc.dram_tensor("wv", [128, CT * VW], BF16, kind="ExternalInput").ap(),
        nc.dram_tensor("wp", [128, 2 * C], BF16, kind="ExternalInput").ap(),
        nc.dram_tensor("bqk", [128, 4], F32, kind="ExternalInput").ap(),
        nc.dram_tensor("bv", [128, VW], F32, kind="ExternalInput").ap(),
        nc.dram_tensor("bp", [128, 8], F32, kind="ExternalInput").ap(),
        nc.dram_tensor("tri", [128, 128], BF16, kind="ExternalInput").ap(),
        nc.dram_tensor("outT", [128, 8 * T], BF16, kind="ExternalOutput").ap(),
    )
    with tile.TileContext(nc) as tc:
        for rep in range(reps):
            if rep:
                tc.strict_bb_all_engine_barrier()
            _emit(tc, nc, d)
    nc.compile()
    return nc


def _sb(a):
    """[128k, n] -> SBUF layout [128, k*n] (k-tile-major along free dim)."""
    k = a.shape[0] // 128
    return np.ascontiguousarray(
        a.reshape(k, 128, a.shape[1]).transpose(1, 0, 2).reshape(128, -1)
    ).astype(np.float32)


def _prep_in_maps(inputs):
    import ml_dtypes
    x = np.asarray(inputs["x"], np.float32)
    W_attn = np.asarray(inputs["W_attn"], np.float32)
    b_attn = np.asarray(inputs["b_attn"], np.float32)
    W_proj = np.asarray(inputs["W_proj"], np.float32)
    b_proj = np.asarray(inputs["b_proj"], np.float32)

    scale = 1.0 / np.sqrt(HD)

    in_maps = []
    for c in range(NCORES):
        b, g = divmod(c, 4)
        heads = [4 * g + i for i in range(HPC)]
        xT = _sb(np.ascontiguousarray(x[b].T)).astype(ml_dtypes.bfloat16)

        wq = [W_attn[:, h * HD:(h + 1) * HD] * scale for h in heads]
        wk = [W_attn[:, C + h * HD:C + (h + 1) * HD] for h in heads]
        wqk = np.concatenate(
            [wq[0], wq[1], wk[0], wk[1], wq[2], wq[3], wk[2], wk[3]], axis=1)
        wqk = _sb(wqk).astype(ml_dtypes.bfloat16)                   # [128, 8*512]

        wv = np.zeros((C, VW), np.float32)
        for i, h in enumerate(heads):
            wv[:, i * 65:i * 65 + 64] = W_attn[:, 2 * C + h * HD:2 * C + (h + 1) * HD]
        wv = _sb(wv).astype(ml_dtypes.bfloat16)                     # [128, 8*260]

        wp = np.zeros((128, 2 * C), np.float32)
        for kt2 in range(2):
            rows = np.concatenate(
                [W_proj[heads[2 * kt2 + j] * HD:(heads[2 * kt2 + j] + 1) * HD, :]
                 for j in range(2)], axis=0)                        # [128, 1024]
            wp[:, kt2 * C:(kt2 + 1) * C] = rows

        wp = wp.astype(ml_dtypes.bfloat16)

        bqk = np.zeros((128, 4), np.float32)
        for i2 in range(2):   # head pair
            for j in range(2):
                h = heads[2 * i2 + j]
                bqk[64 * j:64 * j + 64, 2 * i2] = b_attn[h * HD:(h + 1) * HD] * scale
                bqk[64 * j:64 * j + 64, 2 * i2 + 1] = b_attn[C + h * HD:C + (h + 1) * HD]

        bv = np.zeros(VW, np.float32)
        for i, h in enumerate(heads):
            bv[i * 65:i * 65 + 64] = b_attn[2 * C + h * HD:2 * C + (h + 1) * HD]
            bv[i * 65 + 64] = 1.0
        bv = np.tile(bv[None, :], (128, 1)).astype(np.float32)

        bp = np.zeros((128, 8), np.float32)
        if g == 0:
            bp[:] = b_proj.reshape(8, 128).T

        tri = np.triu(np.ones((128, 128), np.float32)).astype(ml_dtypes.bfloat16)
        in_maps.append({"xT": xT, "wqk": wqk, "wv": wv, "wp": wp,
                        "bqk": bqk, "bv": bv, "bp": bp, "tri": tri})
    return in_maps


def kernel(x, W_attn, b_attn, W_proj, b_proj):
    in_maps = _prep_in_maps(dict(x=x, W_attn=W_attn, b_attn=b_attn,
                                 W_proj=W_proj, b_proj=b_proj))
    if "nc" not in _CACHE:
        _CACHE["nc"] = _build()
    nc = _CACHE["nc"]
    res = run_bass_kernel_spmd(nc, in_maps, core_ids=list(range(NCORES)))

    out = np.zeros((B, T, C), np.float32)
    for c in range(NCORES):
        b = c // 4
        oT = np.asarray(res.results[c]["outT"], dtype=np.float32)   # [128, 8*2048]
        oT = oT.reshape(128, 8, T).transpose(1, 0, 2).reshape(C, T)  # [C, T]
        out[b] += oT.T
    return out


# revision 44
# speedup vs baseline: 1.0012x; 1.0012x over previous
"""Causal self-attention (B=2,T=2048,C=1024,H=16) on 8 trn2 NeuronCores.

Sharding: core c handles batch b=c//4 and 4 heads (c%4)*4..+4 (tensor-parallel
over heads x data-parallel over batch).

Per-core structure: one software-pipelined stream of work units on the PE,
paced so the scalar engine (exp) is fed a score tile roughly every exp-time
while attV / projection matmuls fill the PE between them:

  phase k in 0..3:
    [scores(k) hp0 units x {attV(k-1) subtiles, V-proj(k) tail-half}]
    [scores(k) hp1 units x {qk-proj(k+1), V-proj(k+1) a, out-proj(k-1)}]
  tail: attV(3) subtiles 2-3 interleaved with per-subtile out-proj(3)

Key choices (cost-model driven; all verified on hardware):
  - scores S^T = K_tile' Q_chunk [k-pos, q] in a dedicated 2-deep PSUM pool
    (qk/V/out-proj psums live in a separate pool: scores rotation is never
    stolen); exp -> bf16 es in SBUF, causally trimmed per k-tile; triangular
    mask via gpsimd affine_select on diagonal subtiles only (SBUF-resident:
    gpsimd cannot touch PSUM).
  - attV FLIPPED: stationary = es [128k x 128q] block, moving = V tile
    [128k, 65] bf16 (64 ch + ones col -> softmax denominator in col 64).
    Out y = [q, 4head*65] per q-subtile: 65-cycle matmuls instead of 512.
  - normalize: one strided reciprocal (l cols) + one stride-0-broadcast
    multiply per subtile (DVE); XBAR dma-transpose -> yT [ch, q] bf16
    (no PE transpose, no extra PSUM round-trip).
  - es slots for head-pair 0 are chunk-parity doubled so attV(k-1) floats
    freely among scores(k) hp0 units without write-after-read coupling.
  - x/Wqk/Wv/V/es/yT/Wp all bf16 (PE cost identical at N>=256, halves DMA
    and SBUF; matmul operand dtypes must match 16/32-bit class on HW),
    out f32 accumulate in PSUM, output stored bf16 and upcast on host.
"""
import sys

sys.path.insert(0, "/opt/trn_rl_repo")

import numpy as np

import concourse.bass as bass
import concourse.mybir as mybir
import concourse.tile as tile
from concourse import bacc
from concourse.bass_utils import run_bass_kernel_spmd

B, T, C, H, HD = 2, 2048, 1024, 16, 64
NCORES = 8
HPC = 4            # heads per core
CT = C // 128      # 8 contraction tiles
TJ = T // 512      # 4 tok chunks
TT = T // 128      # 16 tok tiles
VW = HPC * (HD + 1)  # 260: V cols per core incl. ones column per head
F32 = mybir.dt.float32
F32R = mybir.dt.float32r
BF16 = mybir.dt.bfloat16
EXP = mybir.ActivationFunctionType.Exp

_CACHE = {}

# emission-structure knobs (swept experimentally; cost model is the metric)
CFG = {
    "order": "wmix",      # "blocks" = sequential phase blocks, "wmix" = paced interleave
    "attv_pre": "both",    # "hp0": attV hp1 goes into fillers; "both": all attV in pre-block
    "norm_eng": "vector",  # engine for the normalize multiply
    "oproj_eng": "pool",   # engine(s) for out-proj PSUM->SBUF moves
}


def _wmix(s_units, f_units):
    """Spread weighted fillers among score units by cumulative PE time.

    s_units: list of closures; f_units: list of (closure, pe_ns). After each
    score unit, emit fillers until their cumulative time reaches the uniform
    per-score share. Keeps the exp engine fed at a steady cadence.
    """
    if not s_units:
        return [f for f, _ in f_units]
    total = sum(w for _, w in f_units)
    out, fi, acc, target = [], 0, 0.0, 0.0
    for s in s_units:
        out.append(s)
        target += total / len(s_units)
        while fi < len(f_units) and acc < target - 1e-9:
            out.append(f_units[fi][0])
            acc += f_units[fi][1]
            fi += 1
    out.extend(f for f, _ in f_units[fi:])
    return out


def _emit(tc, nc, d):
    d_xT, d_wqk, d_wv, d_wp, d_bqk, d_bv, d_bp, d_out = d
    d_xT3 = d_xT.rearrange("p (c t) -> p c t", c=CT)
    d_out3 = d_out.rearrange("p (m t) -> p m t", m=8)

    with tc.tile_pool(name="const", bufs=1) as pc, \
         tc.tile_pool(name="w", bufs=1) as pw, \
         tc.tile_pool(name="x", bufs=2) as px, \
         tc.tile_pool(name="qk", bufs=1) as pqk, \
         tc.tile_pool(name="vv", bufs=1) as pvv, \
         tc.tile_pool(name="es", bufs=1) as pes, \
         tc.tile_pool(name="yt", bufs=1) as pyt, \
         tc.tile_pool(name="yb", bufs=4) as pyb, \
         tc.tile_pool(name="rc", bufs=4) as prc, \
         tc.tile_pool(name="o", bufs=3) as po, \
         tc.tile_pool(name="o3", bufs=1) as po3, \
         tc.tile_pool(name="psS", bufs=2, space="PSUM") as psS, \
         tc.tile_pool(name="psY", bufs=2, space="PSUM") as psY, \
         tc.tile_pool(name="psC", bufs=2, space="PSUM") as psC:

        bqk = pc.tile([128, 4], F32, tag="bqk")
        bv = pc.tile([128, VW], F32, tag="bv")
        bp = pc.tile([128, 8], F32, tag="bp")
        wp = pc.tile([128, 2 * C], BF16, tag="wp")
        wqk = pw.tile([128, CT * 512], BF16, tag="wqk")
        wv = pw.tile([128, CT * VW], BF16, tag="wv")

        qkT = [pqk.tile([128, T], F32R, tag=f"qk{i}", name=f"qkT{i}")
               for i in range(4)]
        V = [pvv.tile([128, VW], BF16, tag=f"v{i}", name=f"V{i}")
             for i in range(TT)]
        yT = [pyt.tile([128, T], BF16, tag=f"y{i}", name=f"yT{i}")
              for i in range(2)]

        # ---- preamble DMAs: wqk + x(tj0) interleaved ----
        xt = [None] * TJ
        xt[0] = px.tile([128, CT * 512], BF16, tag="xt", name="xt0")
        xt0_3 = xt[0].rearrange("p (c t) -> p c t", c=CT)
        for ct in range(2):
            nc.sync.dma_start(wqk[:, ct * 512:(ct + 1) * 512],
                              d_wqk[:, ct * 512:(ct + 1) * 512])
            nc.sync.dma_start(xt0_3[:, ct:ct + 1, :],
                              d_xT3[:, ct:ct + 1, 0:512])
        for cp in range(1, 4):
            nc.sync.dma_start(wqk[:, cp * 1024:(cp + 1) * 1024],
                              d_wqk[:, cp * 1024:(cp + 1) * 1024])
            nc.sync.dma_start(xt0_3[:, 2 * cp:2 * cp + 2, :],
                              d_xT3[:, 2 * cp:2 * cp + 2, 0:512])
        xt_preloaded = True
        nc.sync.dma_start(bqk[:], d_bqk)
        for h2 in range(2):
            nc.sync.dma_start(wv[:, h2 * 4 * VW:(h2 + 1) * 4 * VW],
                              d_wv[:, h2 * 4 * VW:(h2 + 1) * 4 * VW])
        nc.sync.dma_start(bv[:], d_bv)
        nc.sync.dma_start(wp[:], d_wp)
        nc.sync.dma_start(bp[:], d_bp)

        es_h = {}   # (head, pair) -> es tile of the current chunk
        nmove = [0]  # rotating engine picker for PSUM->SBUF moves

    # ---------------- unit builders (each returns a closure) -------------
        def u_qk(k, mo):
            def run():
                s = psC.tile([128, 512], F32, tag="pp")
                for ct in range(CT):
                    nc.tensor.matmul(
                        s[:],
                        wqk[:, ct * 512 + mo * 128:ct * 512 + (mo + 1) * 128],
                        xt[k][:, ct * 512:(ct + 1) * 512],
                        start=(ct == 0), stop=(ct == CT - 1))
                nc.vector.tensor_scalar_add(
                    qkT[mo][:, k * 512:(k + 1) * 512], s[:], bqk[:, mo:mo + 1])
            return run

        def u_v(k, lt):
            def run():
                s = psC.tile([128, 512], F32, tag="pp")
                for ct in range(CT):
                    nc.tensor.matmul(
                        s[:, 0:VW],
                        xt[k][:, ct * 512 + lt * 128:ct * 512 + (lt + 1) * 128],
                        wv[:, ct * VW:(ct + 1) * VW],
                        start=(ct == 0), stop=(ct == CT - 1))
                nc.vector.tensor_add(V[4 * k + lt][:], s[:, 0:VW], bv[:])
            return run

        def u_score(qj, hp, p, lh):
            def run():
                h = 2 * hp + lh
                lo = 64 * lh
                qt_t, kt_t = qkT[2 * hp], qkT[2 * hp + 1]
                s = psS.tile([128, 1024], F32, tag="s")
                par = qj % 2 if h < 2 else 0
                es = pes.tile([128, 1024], BF16, tag=f"es{h}_{p}_{par}",
                              name=f"es{h}_{p}_{qj}")
                es_h[(h, p, par)] = es
                for ki in range(2):
                    kti = 2 * p + ki
                    dg = kti - 4 * qj
                    so = 0 if dg < 0 else min(dg, 2) * 128
                    nc.tensor.matmul(
                        s[:, ki * 512 + so:(ki + 1) * 512],
                        kt_t[lo:lo + 64, kti * 128:(kti + 1) * 128],
                        qt_t[lo:lo + 64, qj * 512 + so:(qj + 1) * 512],
                        start=True, stop=True)
                if 2 * p + 1 < 4 * qj:
                    nc.scalar.activation(es[:], s[:], EXP)
                else:
                    for ki in range(2):
                        kti = 2 * p + ki
                        dg = kti - 4 * qj
                        eo = max(dg, 0) * 128
                        nc.scalar.activation(
                            es[:, ki * 512 + eo:(ki + 1) * 512],
                            s[:, ki * 512 + eo:(ki + 1) * 512], EXP)
                        if dg >= 0:
                            nc.gpsimd.affine_select(
                                out=es[:, ki * 512 + dg * 128:
                                       ki * 512 + (dg + 1) * 128],
                                in_=es[:, ki * 512 + dg * 128:
                                       ki * 512 + (dg + 1) * 128],
                                compare_op=mybir.AluOpType.is_ge,
                                fill=0.0, base=0,
                                pattern=[[1, 128]],
                                channel_multiplier=-1)
            return run

        def u_attv(qj, sub):
            def run():
                qt_g = 4 * qj + sub
                yp = psY.tile([128, VW], F32, tag="yp")
                for h in range(4):
                    par = qj % 2 if h < 2 else 0
                    for kti in range(qt_g + 1):
                        p, ki = divmod(kti, 2)
                        nc.tensor.matmul(
                            yp[:, h * 65:h * 65 + 65],
                            es_h[(h, p, par)][:, ki * 512 + sub * 128:
                                              ki * 512 + (sub + 1) * 128],
                            V[kti][:, h * 65:h * 65 + 65],
                            start=(kti == 0), stop=(kti == qt_g))
                rc = prc.tile([128, 4], F32, tag="rc")
                nc.vector.reciprocal(rc[:], yp[:, 64:VW:65])
                yb = pyb.tile([128, 256], BF16, tag="yb")
                # one broadcast multiply: [128, 4head x 64ch] x rc (stride-0)
                yp4 = bass.AP(yp.tensor, yp.offset,
                              [[VW, 128], [65, 4], [1, 64]])
                rcb = bass.AP(rc.tensor, rc.offset,
                              [[4, 128], [1, 4], [0, 64]])
                nc.vector.tensor_mul(
                    yb.rearrange("p (h c) -> p h c", h=4), yp4, rcb)
                for hp2 in range(2):
                    nc.sync.dma_start_transpose(
                        yT[hp2][:, qt_g * 128:(qt_g + 1) * 128],
                        yb[:, hp2 * 128:(hp2 + 1) * 128])
            return run

        def u_oproj(qj, mop):
            def run():
                ot = po.tile([128, 1024], BF16, tag="ot")
                for half in range(2):
                    mo = 2 * mop + half
                    pp = psC.tile([128, 512], F32, tag="pp")
                    for kt2 in range(2):
                        nc.tensor.matmul(
                            pp[:],
                            wp[:, kt2 * C + mo * 128:kt2 * C + (mo + 1) * 128],
                            yT[kt2][:, qj * 512:(qj + 1) * 512],
                            start=(kt2 == 0), stop=(kt2 == 1))
                    nc.vector.tensor_scalar_add(
                        ot[:, half * 512:(half + 1) * 512],
                        pp[:], bp[:, mo:mo + 1])
                ot3 = ot.rearrange("p (m t) -> p m t", m=2)
                nc.sync.dma_start(
                    d_out3[:, 2 * mop:2 * mop + 2, qj * 512:(qj + 1) * 512],
                    ot3)
            return run

        # per-subtile out-proj for the last chunk (shrinks the tail)
        ot3_tiles = {}

        def u_oproj3_sub(sub):
            def run():
                qj = TJ - 1
                for mop in range(4):
                    if sub == 0:
                        ot3_tiles[mop] = po3.tile([128, 1024], BF16,
                                                  tag=f"ot3_{mop}",
                                                  name=f"ot3_{mop}")
                    ot = ot3_tiles[mop]
                    pp = psC.tile([128, 512], F32, tag="pp")
                    for half in range(2):
                        mo = 2 * mop + half
                        for kt2 in range(2):
                            nc.tensor.matmul(
                                pp[:, half * 128:(half + 1) * 128],
                                wp[:, kt2 * C + mo * 128:kt2 * C + (mo + 1) * 128],
                                yT[kt2][:, (4 * qj + sub) * 128:
                                        (4 * qj + sub + 1) * 128],
                                start=(kt2 == 0), stop=(kt2 == 1))
                    otv = bass.AP(ot.tensor,
                                  ot.offset + sub * 128,
                                  [[1024, 128], [512, 2], [1, 128]])
                    ppv = pp[:, 0:256].rearrange("p (h c) -> p h c", h=2)
                    bpv = bass.AP(bp.tensor, bp.offset + 2 * mop,
                                  [[8, 128], [1, 2], [0, 128]])
                    nc.vector.tensor_add(otv, ppv, bpv)
                    if sub % 2 == 1:
                        ot3 = ot.rearrange("p (m t) -> p m t", m=2)
                        qa = qj * 512 + (sub - 1) * 128
                        nc.sync.dma_start(
                            d_out3[:, 2 * mop:2 * mop + 2, qa:qa + 256],
                            ot3[:, :, (sub - 1) * 128:(sub + 1) * 128])
            return run

        # ---------------- the phase stream ----------------
        def dma_xt(k):
            if k >= TJ or xt[k] is not None:
                return
            xt[k] = px.tile([128, CT * 512], BF16, tag="xt", name=f"xt{k}x")
            xtn3 = xt[k].rearrange("p (c t) -> p c t", c=CT)
            for cp in range(4):
                nc.sync.dma_start(
                    xtn3[:, 2 * cp:2 * cp + 2, :],
                    d_xT3[:, 2 * cp:2 * cp + 2, k * 512:(k + 1) * 512])

        for k in range(TJ):

            W_QK, W_V, W_OP = 1707.0, 867.0, 854.0

            def w_attv(qj, sub):
                return 108.4 * (4 * qj + sub + 1)

            s0 = [u_score(k, 0, p, lh)
                  for p in range(2 * k + 2) for lh in range(2)]
            s1 = [u_score(k, 1, p, lh)
                  for p in range(2 * k + 2) for lh in range(2)]
            # attV(k, hp0) is ready once s0 completes: it fills s1.
            # attV(k, hp1) becomes the next phase's pre-block.
            fill0 = []
            if k == 0:
                for u in [u_qk(0, 0), u_qk(0, 1)]:
                    u()
                fill0 += [(u_qk(0, 2), W_QK), (u_qk(0, 3), W_QK),
                          (u_v(0, 0), W_V), (u_v(0, 1), W_V)]
            if k >= 1:
                fill0 += [(u_attv(k - 1, s), w_attv(k - 1, s))
                          for s in range(4)]
            fill0 += [(u_v(k, 2), W_V), (u_v(k, 3), W_V)]
            fill1 = []
            nxt = ([(u_qk(k + 1, mo), W_QK) for mo in range(4)]
                   + [(u_v(k + 1, 0), W_V), (u_v(k + 1, 1), W_V)]
                   if k < TJ - 1 else [])
            ops = ([(u_oproj(k - 1, mop), W_OP) for mop in range(4)]
                   if k >= 1 else [])
            # spread oproj among next-phase fillers, oproj not first
            while nxt or ops:
                if len(nxt) >= len(ops) and nxt:
                    fill1.append(nxt.pop(0))
                elif ops:
                    fill1.append(ops.pop(0))

            dma_xt(k + 1)
            for u in _wmix(s0, fill0):
                u()
            tail_av = []
            if k == TJ - 1:
                # attV(3,0/1) right after the s1 units they depend on
                mix = _wmix(s1, fill1)
                n_dep0 = 0
                seen = 0
                out_units = []
                for u in mix:
                    out_units.append(u)
                for u in out_units:
                    u()
                u_attv(k, 0)()
                u_attv(k, 1)()
            else:
                for u in _wmix(s1, fill1):
                    u()

        # ---------------- tail ----------------
        for u in [u_attv(TJ - 1, 2), u_oproj3_sub(0),
                  u_attv(TJ - 1, 3), u_oproj3_sub(1),
                  u_oproj3_sub(2), u_oproj3_sub(3)]:
            u()


def _build(reps=1):
    nc = bacc.Bacc("TRN2", target_bir_lowering=False, debug=False)
    d = (
        nc.dram_tensor("xT", [128, CT * T], BF16, kind="ExternalInput").ap(),
        nc.dram_tensor("wqk", [128, CT * 512], BF16, kind="ExternalInput").ap(),
        nc.dram_tensor("wv", [128, CT * VW], BF16, kind="ExternalInput").ap(),
        nc.dram_tensor("wp", [128, 2 * C], BF16, kind="ExternalInput").ap(),
        nc.dram_tensor("bqk", [128, 4], F32, kind="ExternalInput").ap(),
        nc.dram_tensor("bv", [128, VW], F32, kind="ExternalInput").ap(),
        nc.dram_tensor("bp", [128, 8], F32, kind="ExternalInput").ap(),
        nc.dram_tensor("outT", [128, 8 * T], BF16, kind="ExternalOutput").ap(),
    )
    with tile.TileContext(nc) as tc:
        for rep in range(reps):
            if rep:
                tc.strict_bb_all_engine_barrier()
            _emit(tc, nc, d)
    nc.compile()
    return nc


def _sb(a):
    """[128k, n] -> SBUF layout [128, k*n] (k-tile-major along free dim)."""
    k = a.shape[0] // 128
    return np.ascontiguousarray(
        a.reshape(k, 128, a.shape[1]).transpose(1, 0, 2).reshape(128, -1)
    ).astype(np.float32)


def _prep_in_maps(inputs):
    import ml_dtypes
    x = np.asarray(inputs["x"], np.float32)
    W_attn = np.asarray(inputs["W_attn"], np.float32)
    b_attn = np.asarray(inputs["b_attn"], np.float32)
    W_proj = np.asarray(inputs["W_proj"], np.float32)
    b_proj = np.asarray(inputs["b_proj"], np.float32)

    scale = 1.0 / np.sqrt(HD)

    in_maps = []
    for c in range(NCORES):
        b, g = divmod(c, 4)
        heads = [4 * g + i for i in range(HPC)]
        xT = _sb(np.ascontiguousarray(x[b].T)).astype(ml_dtypes.bfloat16)

        wq = [W_attn[:, h * HD:(h + 1) * HD] * scale for h in heads]
        wk = [W_attn[:, C + h * HD:C + (h + 1) * HD] for h in heads]
        wqk = np.concatenate(
            [wq[0], wq[1], wk[0], wk[1], wq[2], wq[3], wk[2], wk[3]], axis=1)
        wqk = _sb(wqk).astype(ml_dtypes.bfloat16)                   # [128, 8*512]

        wv = np.zeros((C, VW), np.float32)
        for i, h in enumerate(heads):
            wv[:, i * 65:i * 65 + 64] = W_attn[:, 2 * C + h * HD:2 * C + (h + 1) * HD]
        wv = _sb(wv).astype(ml_dtypes.bfloat16)                     # [128, 8*260]

        wp = np.zeros((128, 2 * C), np.float32)
        for kt2 in range(2):
            rows = np.concatenate(
                [W_proj[heads[2 * kt2 + j] * HD:(heads[2 * kt2 + j] + 1) * HD, :]
                 for j in range(2)], axis=0)                        # [128, 1024]
            wp[:, kt2 * C:(kt2 + 1) * C] = rows

        wp = wp.astype(ml_dtypes.bfloat16)

        bqk = np.zeros((128, 4), np.float32)
        for i2 in range(2):   # head pair
            for j in range(2):
                h = heads[2 * i2 + j]
                bqk[64 * j:64 * j + 64, 2 * i2] = b_attn[h * HD:(h + 1) * HD] * scale
                bqk[64 * j:64 * j + 64, 2 * i2 + 1] = b_attn[C + h * HD:C + (h + 1) * HD]

        bv = np.zeros(VW, np.float32)
        for i, h in enumerate(heads):
            bv[i * 65:i * 65 + 64] = b_attn[2 * C + h * HD:2 * C + (h + 1) * HD]
            bv[i * 65 + 64] = 1.0
        bv = np.tile(bv[None, :], (128, 1)).astype(np.float32)

        bp = np.zeros((128, 8), np.float32)
        if g == 0:
            bp[:] = b_proj.reshape(8, 128).T

        in_maps.append({"xT": xT, "wqk": wqk, "wv": wv, "wp": wp,
                        "bqk": bqk, "bv": bv, "bp": bp})
    return in_maps


def kernel(x, W_attn, b_attn, W_proj, b_proj):
    in_maps = _prep_in_maps(dict(x=x, W_attn=W_attn, b_attn=b_attn,
                                 W_proj=W_proj, b_proj=b_proj))
    if "nc" not in _CACHE:
        _CACHE["nc"] = _build()
    nc = _CACHE["nc"]
    res = run_bass_kernel_spmd(nc, in_maps, core_ids=list(range(NCORES)))

    out = np.zeros((B, T, C), np.float32)
    for c in range(NCORES):
        b = c // 4
        oT = np.asarray(res.results[c]["outT"], dtype=np.float32)   # [128, 8*2048]
        oT = oT.reshape(128, 8, T).transpose(1, 0, 2).reshape(C, T)  # [C, T]
        out[b] += oT.T
    return out
